# revision 19
# baseline (speedup 1.0000x reference)
"""Self-contained Trainium2 Bass kernel for nn_ComplementarityScoreHead.

out = (h_norm @ h_norm.T) * edge_mask, h = MLP(x), h_norm = h / ||h||_2(rows)

Strategy (8 NeuronCores, SPMD, no collectives):
  - Each core m receives xT (x transposed on host, bf16) rolled by -1024*m
    nodes, so its 1024-row output slab sits at local columns [0:1024) of the
    on-chip transposed feature matrix; one identical program runs everywhere.
  - Device computes only the DENSE unnormalized correlation h @ h.T in bf16:
    MLP layer1 (bf16 matmul, relu+bias fused into PSUM eviction on
    Act/DVE), layer2 (fp32r matmuls, +bias eviction on Pool/Act/DVE) into a
    transposed bf16 feature matrix hnT, then per 128-row chunk x 1024-col
    window bf16 correlation matmuls; PSUM evictions (plain copies) rotate
    across DVE/Pool/Act and dense bf16 tiles stream to DRAM mostly on SP.
    Correlation windows are interleaved between MLP tiles so all engines
    stay busy; PE (75us of matmul) is the critical resource.
  - Host glue: transpose+quantize x, run cores, compute row norms from the
    returned h slab, then scatter rsq[i]*rsq[j]*corr[i,j] at the ~262k edge
    positions into a zero f32 [8192,8192] output (the edge mask and the
    normalization never touch the device).
"""
import sys
import numpy as np

sys.path.insert(0, '/opt/trn_rl_repo')

import concourse.bass as bass  # noqa: E402
import concourse.mybir as mybir  # noqa: E402
from concourse import bacc  # noqa: E402
from concourse.tile import TileContext  # noqa: E402
from concourse.bass_utils import run_bass_kernel_spmd  # noqa: E402

N = 8192
F = 128
H = 256
NCORES = 8
SLAB = N // NCORES
CHUNKS = SLAB // 128   # 8 chunks of 128 output rows
NT = N // 512          # 16 MLP column tiles
NWIN = N // 1024       # 8 correlation column windows

BF16 = mybir.dt.np(mybir.dt.bfloat16)


def _build_nc():
    f32 = mybir.dt.float32
    f32r = mybir.dt.float32r
    bf16 = mybir.dt.bfloat16
    Relu = mybir.ActivationFunctionType.Relu
    add = mybir.AluOpType.add
    amax = mybir.AluOpType.max

    nc = bacc.Bacc()
    xT = nc.declare_dram_parameter("xT", [F, N], bf16, isOutput=False)
    W1 = nc.declare_dram_parameter("W1", [F, H], bf16, isOutput=False)
    b1 = nc.declare_dram_parameter("b1", [128, 2], f32, isOutput=False)
    W2 = nc.declare_dram_parameter("W2", [128, 2, H], f32, isOutput=False)
    b2 = nc.declare_dram_parameter("b2", [128, 2], f32, isOutput=False)
    out = nc.declare_dram_parameter("out", [SLAB, N], bf16, isOutput=True)
    hout = nc.declare_dram_parameter("hout", [128, 2, SLAB], bf16,
                                     isOutput=True)

    with TileContext(nc) as tc:
        with (
            tc.tile_pool(name="singles", bufs=1) as singles,
            tc.tile_pool(name="hn", bufs=2) as hn_pool,
            tc.tile_pool(name="mid", bufs=3) as mid,
            tc.tile_pool(name="ch", bufs=6) as ch_pool,
            tc.tile_pool(name="psA", bufs=2, space="PSUM") as psA,
            tc.tile_pool(name="psW", bufs=6, space="PSUM") as psW,
        ):
            w1b = singles.tile([128, H], bf16)
            nc.sync.dma_start(out=w1b[:], in_=W1[:])
            b1s = singles.tile([128, 2], f32)
            nc.scalar.dma_start(out=b1s[:], in_=b1[:])
            xts = singles.tile([128, N], bf16)
            for q in range(8):
                qsl = slice(q * 1024, (q + 1) * 1024)
                dmae = nc.sync if q % 2 == 0 else nc.scalar
                dmae.dma_start(out=xts[:, qsl], in_=xT[:, qsl])
            w2f = singles.tile([128, 2, H], f32)
            nc.gpsimd.dma_start(out=w2f[:], in_=W2[:])
            w2r = singles.tile([128, 2, H], f32r)
            nc.vector.tensor_copy(w2r[:], w2f[:])
            b2s = singles.tile([128, 2], f32)
            nc.gpsimd.dma_start(out=b2s[:], in_=b2[:])

            hnT = [hn_pool.tile([128, N], bf16, tag="hn", name=f"hnT{s}")
                   for s in range(2)]

            Ident = mybir.ActivationFunctionType.Identity

            def _copy_act(dst, src):
                nc.scalar.activation(dst, src, Ident)

            # only DVE and Act may read PSUM; Pool/SP carry the output DMA
            ev2 = [lambda d, s: nc.vector.tensor_copy(d, s),
                   _copy_act]
            dma2 = [nc.sync, nc.gpsimd]
            evi = [0]
            dqi = [0]

            def mlp(t):
                sl = slice(t * 512, (t + 1) * 512)
                r1s = mid.tile([128, 2, 512], f32r, tag="r1s")
                for s in range(2):
                    ps = psA.tile([128, 512], f32, tag="ps")
                    nc.tensor.matmul(
                        ps[:], w1b[:, s * 128:(s + 1) * 128], xts[:, sl],
                        start=True, stop=True)
                    if s == 0:
                        nc.scalar.activation(r1s[:, 0, :], ps[:], Relu,
                                             bias=b1s[:, 0:1])
                    else:
                        nc.vector.tensor_scalar(
                            r1s[:, 1, :], ps[:], b1s[:, 1:2], 0.0,
                            op0=add, op1=amax)
                for s2 in range(2):
                    ps = psA.tile([128, 512], f32, tag="ps")
                    for k in range(2):
                        nc.tensor.matmul(
                            ps[:], w2r[:, k, s2 * 128:(s2 + 1) * 128],
                            r1s[:, k, :], start=(k == 0), stop=(k == 1))
                    if s2 == 0:
                        nc.vector.tensor_scalar_add(hnT[0][:, sl], ps[:],
                                                    b2s[:, 0:1])
                    else:
                        nc.scalar.activation(
                            hnT[1][:, sl], ps[:],
                            mybir.ActivationFunctionType.Identity,
                            bias=b2s[:, 1:2])

            def corr(w):
                for mt in range(CHUNKS):
                    for sub in range(2):
                        csl = slice(w * 1024 + sub * 512,
                                    w * 1024 + (sub + 1) * 512)
                        psw = psW.tile([128, 512], f32, tag="psw")
                        for k in range(2):
                            nc.tensor.matmul(
                                psw[:], hnT[k][:, mt * 128:(mt + 1) * 128],
                                hnT[k][:, csl], start=(k == 0), stop=(k == 1))
                        ch = ch_pool.tile([128, 512], bf16, tag="ch")
                        ev2[evi[0] % 2](ch[:], psw[:])
                        evi[0] += 1
                        dq = dma2[dqi[0] % 2]
                        dqi[0] += 1
                        dq.dma_start(
                            out=out[mt * 128:(mt + 1) * 128, csl], in_=ch[:])

            mlp(0)
            mlp(1)
            mlp(2)
            mlp(3)
            for s in range(2):
                nc.sync.dma_start(out=hout[:, s, :], in_=hnT[s][:, 0:SLAB])
            corr(0)
            for t in range(4, NT):
                mlp(t)
                if t % 2 == 1:
                    corr((t - 3) // 2)
            corr(NWIN - 1)
    nc.compile()
    return nc


_NC_CACHE = {}


def get_nc():
    if "nc" not in _NC_CACHE:
        _NC_CACHE["nc"] = _build_nc()
    return _NC_CACHE["nc"]


def prep_in_maps(x, edge_index, W1, b1, W2, b2):
    x = np.asarray(x, dtype=np.float32)
    W1b = np.ascontiguousarray(np.asarray(W1, dtype=np.float32)).astype(BF16)
    W2h = np.ascontiguousarray(
        np.asarray(W2, dtype=np.float32).reshape(2, 128, H).transpose(1, 0, 2))
    b1h = np.ascontiguousarray(np.asarray(b1, dtype=np.float32).reshape(2, 128).T)
    b2h = np.ascontiguousarray(np.asarray(b2, dtype=np.float32).reshape(2, 128).T)
    in_maps = []
    for m in range(NCORES):
        xTm = np.ascontiguousarray(
            np.roll(x, -SLAB * m, axis=0).T.astype(BF16))
        in_maps.append({"xT": xTm, "W1": W1b, "b1": b1h, "W2": W2h,
                        "b2": b2h})
    return in_maps


def assemble(results, edge_index):
    # dense[m] = rolled unnormalized h @ h.T slab of core m (bf16)
    dense = [np.asarray(results[m]["out"]) for m in range(NCORES)]
    # reconstruct h rows from per-core local slabs: hout[p, s, j] is
    # channel s*128+p of local node j (global node m*SLAB+j)
    h = np.empty((N, H), dtype=np.float32)
    for m in range(NCORES):
        hm = np.asarray(results[m]["hout"]).astype(np.float32)
        h[m * SLAB:(m + 1) * SLAB] = hm.transpose(2, 1, 0).reshape(SLAB, H)
    norm = np.maximum(np.sqrt((h * h).sum(axis=1)), 1e-12)
    rsq = (1.0 / norm).astype(np.float32)

    r = np.asarray(edge_index[0], dtype=np.int64)
    c = np.asarray(edge_index[1], dtype=np.int64)
    m = r // SLAB
    lr = r - m * SLAB
    lc = (c - m * SLAB) % N
    vals = np.empty(len(r), dtype=np.float32)
    for mm in range(NCORES):
        sel = m == mm
        vals[sel] = dense[mm][lr[sel], lc[sel]].astype(np.float32)
    out = np.zeros((N, N), dtype=np.float32)
    out[r, c] = vals * rsq[r] * rsq[c]
    return out


def kernel(x, edge_index, W1, b1, W2, b2):
    in_maps = prep_in_maps(x, edge_index, W1, b1, W2, b2)
    nc = get_nc()
    res = run_bass_kernel_spmd(nc, in_maps, list(range(NCORES)))
    return assemble(res.results, edge_index)


# revision 24
# speedup vs baseline: 1.4091x; 1.4091x over previous
"""Self-contained Trainium2 Bass kernel for nn_ComplementarityScoreHead.

out = (h_norm @ h_norm.T) * edge_mask, h = MLP(x), h_norm = h / ||h||_2(rows)

Strategy (8 NeuronCores, SPMD, no collectives):
  - Each core m receives xT (x transposed on host, bf16) rolled by -1024*m
    nodes, so its 1024-row output slab sits at local columns [0:1024) of the
    on-chip transposed feature matrix; one identical program runs everywhere.
  - Device computes only the DENSE unnormalized correlation h @ h.T in bf16:
    MLP layer1 (bf16 matmul, relu+bias fused into PSUM eviction on
    Act/DVE), layer2 (fp32r matmuls, +bias eviction on Pool/Act/DVE) into a
    transposed bf16 feature matrix hnT, then per 128-row chunk x 1024-col
    window bf16 correlation matmuls; PSUM evictions (plain copies) rotate
    across DVE/Pool/Act and dense bf16 tiles stream to DRAM mostly on SP.
    Correlation windows are interleaved between MLP tiles so all engines
    stay busy; PE (75us of matmul) is the critical resource.
  - Host glue: transpose+quantize x, run cores, compute row norms from the
    returned h slab, then scatter rsq[i]*rsq[j]*corr[i,j] at the ~262k edge
    positions into a zero f32 [8192,8192] output (the edge mask and the
    normalization never touch the device).
"""
import sys
import numpy as np

sys.path.insert(0, '/opt/trn_rl_repo')

import concourse.bass as bass  # noqa: E402
import concourse.mybir as mybir  # noqa: E402
from concourse import bacc  # noqa: E402
from concourse.tile import TileContext  # noqa: E402
from concourse.bass_utils import run_bass_kernel_spmd  # noqa: E402

N = 8192
F = 128
H = 256
NCORES = 8
SLAB = N // NCORES
CHUNKS = SLAB // 128   # 8 chunks of 128 output rows
NT = N // 512          # 16 MLP column tiles
NWIN = N // 1024       # 8 correlation column windows

BF16 = mybir.dt.np(mybir.dt.bfloat16)


def _build_nc():
    f32 = mybir.dt.float32
    f32r = mybir.dt.float32r
    bf16 = mybir.dt.bfloat16
    Relu = mybir.ActivationFunctionType.Relu
    add = mybir.AluOpType.add
    amax = mybir.AluOpType.max

    nc = bacc.Bacc()
    xT = nc.declare_dram_parameter("xT", [F, N], bf16, isOutput=False)
    W1 = nc.declare_dram_parameter("W1", [F, H], bf16, isOutput=False)
    b1 = nc.declare_dram_parameter("b1", [128, 2], f32, isOutput=False)
    W2 = nc.declare_dram_parameter("W2", [128, 2, H], f32, isOutput=False)
    b2 = nc.declare_dram_parameter("b2", [128, 2], f32, isOutput=False)
    out = nc.declare_dram_parameter("out", [SLAB, N], bf16, isOutput=True)
    hout = nc.declare_dram_parameter("hout", [128, 2, SLAB], bf16,
                                     isOutput=True)

    with TileContext(nc) as tc:
        with (
            tc.tile_pool(name="singles", bufs=1) as singles,
            tc.tile_pool(name="hn", bufs=2) as hn_pool,
            tc.tile_pool(name="mid", bufs=4) as mid,
            tc.tile_pool(name="ch", bufs=8) as ch_pool,
            tc.tile_pool(name="psA", bufs=3, space="PSUM") as psA,
            tc.tile_pool(name="psW", bufs=5, space="PSUM") as psW,
        ):
            w1b = singles.tile([128, H], bf16)
            nc.sync.dma_start(out=w1b[:], in_=W1[:])
            b1s = singles.tile([128, 2], f32)
            nc.scalar.dma_start(out=b1s[:], in_=b1[:])
            xts = singles.tile([128, N], bf16)
            for q in range(8):
                qsl = slice(q * 1024, (q + 1) * 1024)
                dmae = nc.sync if q % 2 == 0 else nc.scalar
                dmae.dma_start(out=xts[:, qsl], in_=xT[:, qsl])
            w2f = singles.tile([128, 2, H], f32)
            nc.gpsimd.dma_start(out=w2f[:], in_=W2[:])
            w2r = singles.tile([128, 2, H], f32r)
            nc.vector.tensor_copy(w2r[:], w2f[:])
            b2s = singles.tile([128, 2], f32)
            nc.gpsimd.dma_start(out=b2s[:], in_=b2[:])

            hnT = [hn_pool.tile([128, N], bf16, tag="hn", name=f"hnT{s}")
                   for s in range(2)]

            Ident = mybir.ActivationFunctionType.Identity

            def _copy_act(dst, src):
                nc.scalar.activation(dst, src, Ident)

            # only DVE and Act may read PSUM; Pool/SP carry the output DMA
            ev2 = [lambda d, s: nc.vector.tensor_copy(d, s),
                   _copy_act]
            dma2 = [nc.sync, nc.gpsimd]
            evi = [0]
            dqi = [0]

            def mlp(t):
                sl = slice(t * 512, (t + 1) * 512)
                r1s = mid.tile([128, 2, 512], f32r, tag="r1s")
                for s in range(2):
                    ps = psA.tile([128, 512], f32, tag="ps")
                    nc.tensor.matmul(
                        ps[:], w1b[:, s * 128:(s + 1) * 128], xts[:, sl],
                        start=True, stop=True)
                    if s == 0:
                        nc.scalar.activation(r1s[:, 0, :], ps[:], Relu,
                                             bias=b1s[:, 0:1])
                    else:
                        nc.vector.tensor_scalar(
                            r1s[:, 1, :], ps[:], b1s[:, 1:2], 0.0,
                            op0=add, op1=amax)
                for s2 in range(2):
                    ps = psA.tile([128, 512], f32, tag="ps")
                    for k in range(2):
                        nc.tensor.matmul(
                            ps[:], w2r[:, k, s2 * 128:(s2 + 1) * 128],
                            r1s[:, k, :], start=(k == 0), stop=(k == 1))
                    if s2 == 0:
                        nc.vector.tensor_scalar_add(hnT[0][:, sl], ps[:],
                                                    b2s[:, 0:1])
                    else:
                        nc.scalar.activation(
                            hnT[1][:, sl], ps[:],
                            mybir.ActivationFunctionType.Identity,
                            bias=b2s[:, 1:2])

            def corr(w, half=None):
                chunks = range(CHUNKS) if half is None else (
                    range(0, 4) if half == 0 else range(4, CHUNKS))
                for mt in chunks:
                    for sub in range(2):
                        csl = slice(w * 1024 + sub * 512,
                                    w * 1024 + (sub + 1) * 512)
                        psw = psW.tile([128, 512], f32, tag="psw")
                        for k in range(2):
                            nc.tensor.matmul(
                                psw[:], hnT[k][:, mt * 128:(mt + 1) * 128],
                                hnT[k][:, csl], start=(k == 0), stop=(k == 1))
                        ch = ch_pool.tile([128, 512], bf16, tag="ch")
                        ev2[evi[0] % 2](ch[:], psw[:])
                        evi[0] += 1
                        dq = dma2[dqi[0] % 2]
                        dqi[0] += 1
                        dq.dma_start(
                            out=out[mt * 128:(mt + 1) * 128, csl], in_=ch[:])

            mlp(0)
            mlp(1)
            mlp(2)
            for s in range(2):
                nc.sync.dma_start(out=hout[:, s, :], in_=hnT[s][:, 0:SLAB])
            corr(0, 0)
            mlp(3)
            corr(0, 1)
            for t in range(4, NT):
                mlp(t)
                corr((t - 2) // 2, t % 2)
            corr(NWIN - 1)
    nc.compile()
    return nc


_NC_CACHE = {}


def get_nc():
    if "nc" not in _NC_CACHE:
        _NC_CACHE["nc"] = _build_nc()
    return _NC_CACHE["nc"]


def prep_in_maps(x, edge_index, W1, b1, W2, b2):
    x = np.asarray(x, dtype=np.float32)
    W1b = np.ascontiguousarray(np.asarray(W1, dtype=np.float32)).astype(BF16)
    W2h = np.ascontiguousarray(
        np.asarray(W2, dtype=np.float32).reshape(2, 128, H).transpose(1, 0, 2))
    b1h = np.ascontiguousarray(np.asarray(b1, dtype=np.float32).reshape(2, 128).T)
    b2h = np.ascontiguousarray(np.asarray(b2, dtype=np.float32).reshape(2, 128).T)
    in_maps = []
    for m in range(NCORES):
        xTm = np.ascontiguousarray(
            np.roll(x, -SLAB * m, axis=0).T.astype(BF16))
        in_maps.append({"xT": xTm, "W1": W1b, "b1": b1h, "W2": W2h,
                        "b2": b2h})
    return in_maps


def assemble(results, edge_index):
    # dense[m] = rolled unnormalized h @ h.T slab of core m (bf16)
    dense = [np.asarray(results[m]["out"]) for m in range(NCORES)]
    # reconstruct h rows from per-core local slabs: hout[p, s, j] is
    # channel s*128+p of local node j (global node m*SLAB+j)
    h = np.empty((N, H), dtype=np.float32)
    for m in range(NCORES):
        hm = np.asarray(results[m]["hout"]).astype(np.float32)
        h[m * SLAB:(m + 1) * SLAB] = hm.transpose(2, 1, 0).reshape(SLAB, H)
    norm = np.maximum(np.sqrt((h * h).sum(axis=1)), 1e-12)
    rsq = (1.0 / norm).astype(np.float32)

    r = np.asarray(edge_index[0], dtype=np.int64)
    c = np.asarray(edge_index[1], dtype=np.int64)
    m = r // SLAB
    lr = r - m * SLAB
    lc = (c - m * SLAB) % N
    vals = np.empty(len(r), dtype=np.float32)
    for mm in range(NCORES):
        sel = m == mm
        vals[sel] = dense[mm][lr[sel], lc[sel]].astype(np.float32)
    out = np.zeros((N, N), dtype=np.float32)
    out[r, c] = vals * rsq[r] * rsq[c]
    return out


def kernel(x, edge_index, W1, b1, W2, b2):
    in_maps = prep_in_maps(x, edge_index, W1, b1, W2, b2)
    nc = get_nc()
    res = run_bass_kernel_spmd(nc, in_maps, list(range(NCORES)))
    return assemble(res.results, edge_index)


# revision 28
# speedup vs baseline: 1.8436x; 1.3083x over previous
"""Self-contained Trainium2 Bass kernel for nn_ComplementarityScoreHead.

out = (h_norm @ h_norm.T) * edge_mask, h = MLP(x), h_norm = h / ||h||_2(rows)

Strategy (8 NeuronCores, SPMD, no collectives):
  - Each core m receives xT (x transposed on host, bf16) rolled by -1024*m
    nodes, so its 1024-row output slab sits at local columns [0:1024) of the
    on-chip transposed feature matrix; one identical program runs everywhere.
  - Device computes only the DENSE unnormalized correlation h @ h.T in bf16:
    MLP layer1 (bf16 matmul, relu+bias fused into PSUM eviction on
    Act/DVE), layer2 (fp32r matmuls, +bias eviction on Pool/Act/DVE) into a
    transposed bf16 feature matrix hnT, then per 128-row chunk x 1024-col
    window bf16 correlation matmuls; PSUM evictions (plain copies) rotate
    across DVE/Pool/Act and dense bf16 tiles stream to DRAM mostly on SP.
    Correlation windows are interleaved between MLP tiles so all engines
    stay busy; PE (75us of matmul) is the critical resource.
  - Host glue: transpose+quantize x, run cores, compute row norms from the
    returned h slab, then scatter rsq[i]*rsq[j]*corr[i,j] at the ~262k edge
    positions into a zero f32 [8192,8192] output (the edge mask and the
    normalization never touch the device).
"""
import sys
import numpy as np

sys.path.insert(0, '/opt/trn_rl_repo')

import concourse.bass as bass  # noqa: E402
import concourse.mybir as mybir  # noqa: E402
from concourse import bacc  # noqa: E402
from concourse.tile import TileContext  # noqa: E402
from concourse.bass_utils import run_bass_kernel_spmd  # noqa: E402

N = 8192
F = 128
H = 256
NCORES = 8
SLAB = N // NCORES
CHUNKS = SLAB // 128   # 8 chunks of 128 output rows
NT = N // 512          # 16 MLP column tiles
NWIN = N // 1024       # 8 correlation column windows

BF16 = mybir.dt.np(mybir.dt.bfloat16)


def _build_nc():
    f32 = mybir.dt.float32
    f32r = mybir.dt.float32r
    bf16 = mybir.dt.bfloat16
    Relu = mybir.ActivationFunctionType.Relu
    add = mybir.AluOpType.add
    amax = mybir.AluOpType.max

    nc = bacc.Bacc()
    xT = nc.declare_dram_parameter("xT", [F, N], bf16, isOutput=False)
    W1 = nc.declare_dram_parameter("W1", [F, H], bf16, isOutput=False)
    b1 = nc.declare_dram_parameter("b1", [128, 2], f32, isOutput=False)
    W2 = nc.declare_dram_parameter("W2", [128, 2, H], f32, isOutput=False)
    b2 = nc.declare_dram_parameter("b2", [128, 2], f32, isOutput=False)
    out = nc.declare_dram_parameter("out", [SLAB, N], bf16, isOutput=True)
    hout = nc.declare_dram_parameter("hout", [128, 2, SLAB], bf16,
                                     isOutput=True)

    with TileContext(nc) as tc:
        with (
            tc.tile_pool(name="singles", bufs=1) as singles,
            tc.tile_pool(name="hn", bufs=2) as hn_pool,
            tc.tile_pool(name="mid", bufs=4) as mid,
            tc.tile_pool(name="ch", bufs=8) as ch_pool,
            tc.tile_pool(name="psA", bufs=3, space="PSUM") as psA,
            tc.tile_pool(name="psW", bufs=5, space="PSUM") as psW,
        ):
            # keep Act free of early loads: its queue must reach the first
            # relu evictions quickly (they gate L2 and the first corr window)
            w1b = singles.tile([128, H], bf16)
            nc.sync.dma_start(out=w1b[:], in_=W1[:])
            b1s = singles.tile([128, 2], f32)
            nc.scalar.dma_start(out=b1s[:], in_=b1[:])
            xts = singles.tile([128, N], bf16)
            for q in range(8):
                qsl = slice(q * 1024, (q + 1) * 1024)
                dmae = nc.sync if q % 2 == 0 else nc.gpsimd
                dmae.dma_start(out=xts[:, qsl], in_=xT[:, qsl])
            w2f = singles.tile([128, 2, H], f32)
            nc.gpsimd.dma_start(out=w2f[:], in_=W2[:])
            w2r = singles.tile([128, 2, H], f32r)
            nc.vector.tensor_copy(w2r[:], w2f[:])
            b2s = singles.tile([128, 2], f32)
            nc.gpsimd.dma_start(out=b2s[:], in_=b2[:])

            hnT = [hn_pool.tile([128, N], bf16, tag="hn", name=f"hnT{s}")
                   for s in range(2)]

            Ident = mybir.ActivationFunctionType.Identity

            def _copy_act(dst, src):
                nc.scalar.activation(dst, src, Ident)

            # only DVE and Act may read PSUM; Pool/SP carry the output DMA
            ev2 = [lambda d, s: nc.vector.tensor_copy(d, s),
                   _copy_act]
            dma2 = [nc.sync, nc.gpsimd]
            evi = [0]
            dqi = [0]

            def mlp(t):
                sl = slice(t * 512, (t + 1) * 512)
                r1s = mid.tile([128, 2, 512], f32r, tag="r1s")
                for s in range(2):
                    ps = psA.tile([128, 512], f32, tag="ps")
                    nc.tensor.matmul(
                        ps[:], w1b[:, s * 128:(s + 1) * 128], xts[:, sl],
                        start=True, stop=True)
                    if s == 0:
                        nc.scalar.activation(r1s[:, 0, :], ps[:], Relu,
                                             bias=b1s[:, 0:1])
                    else:
                        nc.vector.tensor_scalar(
                            r1s[:, 1, :], ps[:], b1s[:, 1:2], 0.0,
                            op0=add, op1=amax)
                for s2 in range(2):
                    ps = psA.tile([128, 512], f32, tag="ps")
                    for k in range(2):
                        nc.tensor.matmul(
                            ps[:], w2r[:, k, s2 * 128:(s2 + 1) * 128],
                            r1s[:, k, :], start=(k == 0), stop=(k == 1))
                    if s2 == 0:
                        nc.vector.tensor_scalar_add(hnT[0][:, sl], ps[:],
                                                    b2s[:, 0:1])
                    else:
                        nc.scalar.activation(
                            hnT[1][:, sl], ps[:],
                            mybir.ActivationFunctionType.Identity,
                            bias=b2s[:, 1:2])

            def corr(w, half=None, fine_last=False):
                chunks = range(CHUNKS) if half is None else (
                    range(0, 4) if half == 0 else range(4, CHUNKS))
                for mt in chunks:
                    for sub in range(2):
                        c0 = w * 1024 + sub * 512
                        csl = slice(c0, c0 + 512)
                        psw = psW.tile([128, 512], f32, tag="psw")
                        for k in range(2):
                            nc.tensor.matmul(
                                psw[:], hnT[k][:, mt * 128:(mt + 1) * 128],
                                hnT[k][:, csl], start=(k == 0), stop=(k == 1))
                        ch = ch_pool.tile([128, 512], bf16, tag="ch")
                        last = fine_last and mt == CHUNKS - 1 and sub == 1
                        rsl = mt * 128
                        if last:
                            # drain the final tile in 2x256 strips with
                            # independent tiles so the evictions and DMAs
                            # run in parallel ahead of the fixed DMA latency
                            chb = ch_pool.tile([128, 256], bf16, tag="ch")
                            for p, dst in enumerate((ch, chb)):
                                ev2[(evi[0] + p) % 2](
                                    dst[:, 0:256],
                                    psw[:, p * 256:(p + 1) * 256])
                                dma2[(dqi[0] + p) % 2].dma_start(
                                    out=out[rsl:rsl + 128,
                                            c0 + p * 256:c0 + (p + 1) * 256],
                                    in_=dst[:, 0:256])
                            evi[0] += 1
                            dqi[0] += 1
                            continue
                        ev2[evi[0] % 2](ch[:], psw[:])
                        evi[0] += 1
                        dq = dma2[dqi[0] % 2]
                        dqi[0] += 1
                        dq.dma_start(
                            out=out[rsl:rsl + 128, csl], in_=ch[:])

            mlp(0)
            mlp(1)
            mlp(2)
            for s in range(2):
                nc.sync.dma_start(out=hout[:, s, :], in_=hnT[s][:, 0:SLAB])
            corr(0, 0)
            mlp(3)
            corr(0, 1)
            for t in range(4, NT):
                mlp(t)
                corr((t - 2) // 2, t % 2)
            corr(NWIN - 1, fine_last=True)
    nc.compile()
    return nc


_NC_CACHE = {}


def get_nc():
    if "nc" not in _NC_CACHE:
        _NC_CACHE["nc"] = _build_nc()
    return _NC_CACHE["nc"]


def prep_in_maps(x, edge_index, W1, b1, W2, b2):
    x = np.asarray(x, dtype=np.float32)
    W1b = np.ascontiguousarray(np.asarray(W1, dtype=np.float32)).astype(BF16)
    W2h = np.ascontiguousarray(
        np.asarray(W2, dtype=np.float32).reshape(2, 128, H).transpose(1, 0, 2))
    b1h = np.ascontiguousarray(np.asarray(b1, dtype=np.float32).reshape(2, 128).T)
    b2h = np.ascontiguousarray(np.asarray(b2, dtype=np.float32).reshape(2, 128).T)
    in_maps = []
    for m in range(NCORES):
        xTm = np.ascontiguousarray(
            np.roll(x, -SLAB * m, axis=0).T.astype(BF16))
        in_maps.append({"xT": xTm, "W1": W1b, "b1": b1h, "W2": W2h,
                        "b2": b2h})
    return in_maps


def assemble(results, edge_index):
    # dense[m] = rolled unnormalized h @ h.T slab of core m (bf16)
    dense = [np.asarray(results[m]["out"]) for m in range(NCORES)]
    # reconstruct h rows from per-core local slabs: hout[p, s, j] is
    # channel s*128+p of local node j (global node m*SLAB+j)
    h = np.empty((N, H), dtype=np.float32)
    for m in range(NCORES):
        hm = np.asarray(results[m]["hout"]).astype(np.float32)
        h[m * SLAB:(m + 1) * SLAB] = hm.transpose(2, 1, 0).reshape(SLAB, H)
    norm = np.maximum(np.sqrt((h * h).sum(axis=1)), 1e-12)
    rsq = (1.0 / norm).astype(np.float32)

    r = np.asarray(edge_index[0], dtype=np.int64)
    c = np.asarray(edge_index[1], dtype=np.int64)
    m = r // SLAB
    lr = r - m * SLAB
    lc = (c - m * SLAB) % N
    vals = np.empty(len(r), dtype=np.float32)
    for mm in range(NCORES):
        sel = m == mm
        vals[sel] = dense[mm][lr[sel], lc[sel]].astype(np.float32)
    out = np.zeros((N, N), dtype=np.float32)
    out[r, c] = vals * rsq[r] * rsq[c]
    return out


def kernel(x, edge_index, W1, b1, W2, b2):
    in_maps = prep_in_maps(x, edge_index, W1, b1, W2, b2)
    nc = get_nc()
    res = run_bass_kernel_spmd(nc, in_maps, list(range(NCORES)))
    return assemble(res.results, edge_index)
